# revision 12
# baseline (speedup 1.0000x reference)
"""Causal multi-head attention (B=2, S=2048, D=1024, 16 heads of 64) on 8 TRN2
NeuronCores.

Sharding: core c -> batch b = c//4, head-group g = c%4 (4 heads = 256 model
dims per core).  Wq/Wk/Wv column-parallel, Wo row-parallel; the 4 partial
outputs per batch are summed on the host (no collectives).

Per-core data flow (matmul compute in bf16, fp32 PSUM accumulation):
  QT = (Wq_g/8) @ x^T      [256, 2048]   (1/sqrt(hd) folded into Wq,bq)
  KT = Wk_g @ x^T          [256, 2048]
  V  = x @ Wv_g^T + bv     [2048, 256]   natural layout, ones-augmented
  attention per head pair:
    ST[sk,sq] = K_h @ Q_h^T          two heads at row groups 0/64
    causal mask applied IN PSUM via a second accumulating matmul
        (identity stationary x (-3e4) upper-triangle moving tile)
    P = exp(ST)                      ACT exp, [128,1024] two heads wide
    AV: single K=128 matmul per (head, ki) accumulating a merged
        [65,1024] pos tile (hh halves side by side; ones-row of V gives l)
    evac: preoutT = pos[0:64] * partition_broadcast(1/l)  (l-row copied to
        a base-partition-0 tile first; custom DVE/gpsimd ops misread
        partition-offset APs on HW)
  out_partial = preoutT.T @ Wo_g^T   [2048, 1024] fp32
Host: out[b] = sum of the 4 head-group partials + bo.

Scheduling notes (from trace analysis of the previous version):
  - each dma_start costs ~600-900ns of sequencer time to configure the DGE;
    inputs are split across the TWO hwdge queues (sync: xT chunks,
    scalar: weights) so configs run in parallel and the first projection
    matmuls chase the per-chunk DMAs instead of waiting ~13us.
  - Q-t0/K-t0 projections are chunk-chased during the input load (Q in
    mm_ps, K in po_ps, 8 PSUM banks total); evacs split DVE/ACT.
  - PSUM pools: 2+2 bufs of [128,1024] fp32 (4KB) each.
"""

import numpy as np
import ml_dtypes

B, S, D = 2, 2048, 1024
HD = 64
NH = D // HD
N_CORES = 8
GROUPS = 4          # head-groups (tensor-parallel)
JG = D // GROUPS    # local dims per core = 256
NHL = JG // HD      # local heads = 4
KCH = D // 128      # contraction chunks for projections = 8
NKT = S // 128      # sk tiles = 16
NJB = S // 512      # query blocks of 512 = 4

BF16 = ml_dtypes.bfloat16

_cached = {}


def _build():
    import concourse.bacc as bacc
    import concourse.tile as tile
    import concourse.mybir as mybir

    f32 = mybir.dt.float32
    bf16 = mybir.dt.bfloat16
    Exp = mybir.ActivationFunctionType.Exp
    Identity = mybir.ActivationFunctionType.Identity

    nc = bacc.Bacc("TRN2", target_bir_lowering=False, debug=False,
                   num_devices=N_CORES)

    xT = nc.dram_tensor("xT", [128, KCH, S], bf16, kind="ExternalInput").ap()
    wqT = nc.dram_tensor("wqT", [128, KCH, JG], bf16, kind="ExternalInput").ap()
    wkT = nc.dram_tensor("wkT", [128, KCH, JG], bf16, kind="ExternalInput").ap()
    wvT = nc.dram_tensor("wvT", [128, KCH, JG], bf16, kind="ExternalInput").ap()
    woT = nc.dram_tensor("woT", [128, 2, D], bf16, kind="ExternalInput").ap()
    bqk = nc.dram_tensor("bqk", [128, 4], f32, kind="ExternalInput").ap()
    bvb = nc.dram_tensor("bvb", [128, JG], f32, kind="ExternalInput").ap()
    cmask = nc.dram_tensor("cmask", [128, 256], bf16, kind="ExternalInput").ap()
    out = nc.dram_tensor("out", [S, D], bf16, kind="ExternalOutput").ap()

    with tile.TileContext(nc) as tc:
        with (
            tc.tile_pool(name="const", bufs=1) as cpool,
            tc.tile_pool(name="pbig", bufs=2) as p_pool,
            tc.tile_pool(name="small", bufs=4) as small_pool,
            tc.tile_pool(name="outp", bufs=3) as out_pool,
            tc.tile_pool(name="mm_ps", bufs=2, space="PSUM") as mm_ps,
            tc.tile_pool(name="po_ps", bufs=2, space="PSUM") as po_ps,
        ):
            # ---- input DMAs, two hwdge queues in consumption order ----
            # scalar queue: weights.  wq/wk split per k-chunk so the
            # chunk-chasing projections only wait on their own chunk.
            wq_sb = cpool.tile([128, KCH, JG], bf16)
            wk_sb = cpool.tile([128, KCH, JG], bf16)
            for k in range(4):
                nc.scalar.dma_start(wq_sb[:, k, :], wqT[:, k, :])
            bqk_sb = cpool.tile([128, 4], f32)
            nc.scalar.dma_start(bqk_sb[:], bqk[:])
            cm_sb = cpool.tile([128, 256], bf16)
            nc.scalar.dma_start(cm_sb[:], cmask[:])
            for k in range(4, KCH):
                nc.scalar.dma_start(wq_sb[:, k, :], wqT[:, k, :])
            for k in range(KCH):
                nc.scalar.dma_start(wk_sb[:, k, :], wkT[:, k, :])
            # sync queue: the big xT chunks, then the late-needed weights
            xt_all = cpool.tile([128, KCH, S], bf16)
            for k in range(KCH):
                nc.sync.dma_start(xt_all[:, k, :], xT[:, k, :])
            wv_sb = cpool.tile([128, KCH, JG], bf16)
            nc.sync.dma_start(wv_sb[:], wvT[:])
            bvb_sb = cpool.tile([128, JG], f32)
            nc.sync.dma_start(bvb_sb[:], bvb[:])
            wo_sb = cpool.tile([128, 2, D], bf16)
            nc.sync.dma_start(wo_sb[:], woT[:])

            qt = [cpool.tile([128, S], bf16, name=f"qt{t}") for t in range(2)]
            kt = [cpool.tile([128, S], bf16, name=f"kt{t}") for t in range(2)]
            v_all = cpool.tile([128, NKT, NHL * 65], bf16)
            nc.vector.memset(
                v_all.rearrange("p k (h c) -> p k h c", c=65)[:, :, :, 64:65], 1.0)
            po = [cpool.tile([128, S], bf16, name=f"po{t}") for t in range(2)]
            ones64 = cpool.tile([1, 64], f32)
            nc.vector.memset(ones64[:], 1.0)
            warm = small_pool.tile([1, 4], f32, tag="lrow")
            nc.vector.memset(warm[:], 0.0)
            nc.scalar.activation(warm[:], warm[:], Exp)

            # ---- load-phase: Q-t0 / K-t0 chunk-chasing ----
            # 4 live [128,1024] PSUM tiles (all 8 banks); emission order is
            # arrival order: Q k=0..3 first (wk configs trail wq's on the
            # scalar queue), then interleaved K/Q.
            q_ps = [mm_ps.tile([128, 1024], f32, tag="mm", name=f"q0ps{n}")
                    for n in range(2)]
            k_ps = [po_ps.tile([128, 1024], f32, tag="po", name=f"k0ps{n}")
                    for n in range(2)]

            def chase(ps, w_sb_, k):
                for np_ in range(2):
                    for half in range(2):
                        n = 2 * np_ + half
                        nc.tensor.matmul(
                            ps[np_][:, 512 * half:512 * half + 512],
                            lhsT=w_sb_[:, k, 0:128],
                            rhs=xt_all[:, k, 512 * n:512 * n + 512],
                            start=(k == 0), stop=(k == KCH - 1))

            for k in range(4):
                chase(q_ps, wq_sb, k)
            for k in range(4):
                chase(k_ps, wk_sb, k)
                chase(q_ps, wq_sb, k + 4)
            for k in range(4, KCH):
                chase(k_ps, wk_sb, k)
            # evacs: Q on DVE, K on ACT (parallel); np0 first so attention
            # (which needs cols 0:512 of qt/kt) unblocks earliest.
            nc.vector.tensor_scalar_add(qt[0][:, 0:1024], q_ps[0][:],
                                        bqk_sb[:, 0:1])
            nc.scalar.activation(kt[0][:, 0:1024], k_ps[0][:], Identity,
                                 bias=bqk_sb[:, 2:3])
            nc.vector.tensor_scalar_add(qt[0][:, 1024:2048], q_ps[1][:],
                                        bqk_sb[:, 0:1])
            nc.scalar.activation(kt[0][:, 1024:2048], k_ps[1][:], Identity,
                                 bias=bqk_sb[:, 2:3])

            # ---- phase helpers ----

            def proj_qkt(w_sb_, bcol, dst, t):
                for np_ in range(2):
                    ps = mm_ps.tile([128, 1024], f32, tag="mm",
                                    name=f"psproj{t}_{np_}")
                    for k in range(KCH):
                        for half in range(2):
                            n = 2 * np_ + half
                            nc.tensor.matmul(
                                ps[:, 512 * half:512 * half + 512],
                                lhsT=w_sb_[:, k, 128 * t:128 * t + 128],
                                rhs=xt_all[:, k, 512 * n:512 * n + 512],
                                start=(k == 0), stop=(k == KCH - 1))
                    nc.vector.tensor_scalar_add(
                        dst[t][:, 1024 * np_:1024 * np_ + 1024], ps[:],
                        bqk_sb[:, bcol + t:bcol + t + 1])

            def proj_v(sg):
                # two s-tiles per [128,1024] tile, one per bank
                ps = mm_ps.tile([128, 1024], f32, tag="mm", name=f"psv{sg}")
                for k in range(KCH):
                    for q in range(2):
                        si = 2 * sg + q
                        nc.tensor.matmul(
                            ps[:, 512 * q:512 * q + 256],
                            lhsT=xt_all[:, k, 128 * si:128 * si + 128],
                            rhs=wv_sb[:, k, :],
                            start=(k == 0), stop=(k == KCH - 1))
                for q in range(2):
                    si = 2 * sg + q
                    nc.vector.tensor_add(
                        v_all[:, si, :].rearrange(
                            "p (h c) -> p h c", c=65)[:, :, 0:64],
                        ps[:, 512 * q:512 * q + 256].rearrange(
                            "p (h c) -> p h c", c=64),
                        bvb_sb.rearrange("p (h c) -> p h c", c=64))

            def attn_scores(pair, j, interleave=None):
                nk = 4 * (j + 1)
                qt_t, kt_t = qt[pair], kt[pair]
                pt_all = p_pool.tile([128, NKT, 1024], bf16, tag="p",
                                     name=f"pt{pair}_{j}")
                for ki in range(nk):
                    d = max(0, 128 * ki - 512 * j)
                    st = mm_ps.tile([128, 1024], f32, tag="mm",
                                    name=f"st{pair}_{j}_{ki}")
                    diag = ki >= 4 * j
                    for hh in range(2):
                        base = 64 * hh
                        nc.tensor.matmul(
                            st[:, 512 * hh + d:512 * hh + 512],
                            lhsT=kt_t[base:base + 64,
                                      128 * ki:128 * ki + 128],
                            rhs=qt_t[base:base + 64,
                                     512 * j + d:512 * j + 512],
                            start=True, stop=not diag,
                            skip_group_check=diag)
                    if diag:
                        # accumulate -3e4 into the strictly-lower-triangle
                        # (sk>sq) of the 128-wide diagonal sub-block, so exp
                        # gives exactly 0 there: identity (stationary) x
                        # maskneg (moving).
                        for hh in range(2):
                            nc.tensor.matmul(
                                st[:, 512 * hh + d:512 * hh + d + 128],
                                lhsT=cm_sb[:, 0:128],
                                rhs=cm_sb[:, 128:256],
                                start=False, stop=True,
                                skip_group_check=True)
                    if d == 0:
                        nc.scalar.activation(pt_all[:, ki, :], st[:], Exp)
                    else:
                        for hh in range(2):
                            nc.scalar.activation(
                                pt_all[:, ki, 512 * hh + d:512 * hh + 512],
                                st[:, 512 * hh + d:512 * hh + 512], Exp)
                    if interleave is not None:
                        interleave(ki)
                return pt_all

            def attn_av(pair, j, pt_all, final=False):
                nk = 4 * (j + 1)
                # AV: single K=128 matmul per (head, ki) — matmul wall time
                # depends only on the moving size N.  Both heads accumulate
                # into one [65,1024] tile (hh halves side by side).
                pos = po_ps.tile([65, 1024], f32, tag="po",
                                 name=f"pos{pair}_{j}")
                for ki in range(nk):
                    d = max(0, 128 * ki - 512 * j)
                    for hh in range(2):
                        h = 2 * pair + hh
                        nc.tensor.matmul(
                            pos[0:65, 512 * hh + d:512 * hh + 512],
                            lhsT=v_all[:, ki, 65 * h:65 * h + 65],
                            rhs=pt_all[:, ki, 512 * hh + d:512 * hh + 512],
                            start=(ki == 0), stop=(ki == nk - 1),
                            skip_group_check=True)
                # evac: normalize by 1/l. The l-row must be copied to a
                # base-partition-0 tile first — the custom DVE/gpsimd ops
                # (recip, partition_broadcast) misread partition-offset APs
                # on HW.  Ops stay 512-wide per head so the chain pipelines.
                lrow = [small_pool.tile([1, 512], f32, tag="lrow",
                                        name=f"lrow{hh}") for hh in range(2)]
                lrec = [small_pool.tile([1, 512], f32, tag="lrec",
                                        name=f"lrec{hh}") for hh in range(2)]
                for hh in range(2):
                    nc.vector.tensor_copy(lrow[hh][:],
                                          pos[64:65, 512 * hh:512 * hh + 512])
                    nc.vector.reciprocal_approx_fast(lrec[hh][:], lrow[hh][:])
                rb = [small_pool.tile([64, 512], f32, tag="rb",
                                      name=f"rb{hh}") for hh in range(2)]
                for hh in range(2):
                    nc.gpsimd.partition_broadcast(rb[hh][:], lrec[hh][:])
                for hh in range(2):
                    nc.vector.tensor_mul(
                        po[pair][64 * hh:64 * hh + 64,
                                 512 * j:512 * j + 512],
                        pos[0:64, 512 * hh:512 * hh + 512],
                        rb[hh][:])

            def wo_tile(m, evac_engine=None):
                ps = mm_ps.tile([128, 1024], f32, tag="mm", name=f"pswo{m}")
                for t in range(2):
                    for n in range(2):
                        nc.tensor.matmul(
                            ps[:, 512 * n:512 * n + 512],
                            lhsT=po[t][:, 128 * m:128 * m + 128],
                            rhs=wo_sb[:, t, 512 * n:512 * n + 512],
                            start=(t == 0), stop=(t == 1))
                ob = out_pool.tile([128, 1024], bf16, tag="ob")
                if evac_engine == "scalar":
                    nc.scalar.copy(ob[:], ps[:])
                else:
                    nc.vector.tensor_copy(ob[:], ps[:])
                nc.sync.dma_start(out[128 * m:128 * m + 128, :], ob[:])

            def wo_block(wj, alternate=False):
                # scalar-engine evacs only where ACT has slack (the final
                # block); elsewhere they would stall the exp stream.
                for m in range(4 * wj, 4 * wj + 4):
                    wo_tile(m, evac_engine="scalar" if alternate and m % 2
                            else None)

            # ---- schedule: deep software pipeline — the NEXT block's
            # scores (and their exps) are emitted before the CURRENT block's
            # AV, so the ACT engine always has queued exp work and the AV
            # matmuls never wait on a cold exp chain.  V projections and wo
            # blocks slot into the PE stream between scores and AV. ----
            proj_v(0)
            proj_v(1)
            pt00 = attn_scores(0, 0)
            attn_av(0, 0, pt00)
            proj_qkt(wq_sb, 0, qt, 1)
            proj_qkt(wk_sb, 2, kt, 1)
            pt10 = attn_scores(1, 0)
            proj_v(2)
            attn_av(1, 0, pt10)
            pt01 = attn_scores(0, 1)
            proj_v(3)
            attn_av(0, 1, pt01)
            pt11 = attn_scores(1, 1)
            proj_v(4)
            attn_av(1, 1, pt11)
            pt02 = attn_scores(0, 2)
            proj_v(5)
            attn_av(0, 2, pt02)
            pt12 = attn_scores(1, 2)
            wo_block(0)
            attn_av(1, 2, pt12)
            pt03 = attn_scores(0, 3)
            wo_block(1)
            proj_v(6)
            proj_v(7)
            # the last scores block is emitted before av(0,3) so its exps
            # queue behind (0,3)'s on ACT; wo2 tiles keep the PE busy
            # between them
            wo2 = iter(range(8, 12))

            def fill_wo2(ki):
                if ki % 4 == 3:
                    m = next(wo2, None)
                    if m is not None:
                        wo_tile(m)

            pt13 = attn_scores(1, 3, interleave=fill_wo2)
            attn_av(0, 3, pt03)
            attn_av(1, 3, pt13, final=True)
            wo_block(3, alternate=True)

    nc.compile()
    return nc


def _get_nc():
    if "nc" not in _cached:
        _cached["nc"] = _build()
    return _cached["nc"]


def _make_in_maps(x, Wq, bq, Wk, bk, Wv, bv, Wo):
    sc = 1.0 / np.sqrt(HD)
    tri = np.arange(128)
    ident = np.eye(128, dtype=np.float32)
    maskneg = np.where(tri[:, None] > tri[None, :], -30000.0, 0.0)
    cm = np.concatenate([ident, maskneg], axis=1).astype(BF16)
    in_maps = []
    for c in range(N_CORES):
        b, g = divmod(c, GROUPS)
        sl = slice(JG * g, JG * (g + 1))
        def tile_k(a):  # [D, M] -> [128, D//128, M] contiguous
            return np.ascontiguousarray(
                a.reshape(a.shape[0] // 128, 128, a.shape[1]).transpose(1, 0, 2))

        bqs = (bq[sl] * sc).astype(np.float32)
        bks = bk[sl].astype(np.float32)
        in_maps.append({
            "xT": tile_k(x[b].T.astype(BF16)),
            "wqT": tile_k((Wq[sl] * sc).T.astype(BF16)),
            "wkT": tile_k(Wk[sl].T.astype(BF16)),
            "wvT": tile_k(Wv[sl].T.astype(BF16)),
            "woT": tile_k(Wo[:, sl].T.astype(BF16)),
            "bqk": np.stack([bqs[0:128], bqs[128:256],
                             bks[0:128], bks[128:256]], axis=1).copy(),
            "bvb": np.broadcast_to(bv[sl].astype(np.float32), (128, JG)).copy(),
            "cmask": cm,
        })
    return in_maps


def kernel(x, Wq, bq, Wk, bk, Wv, bv, Wo, bo, _return_results=False):
    from concourse.bass_utils import run_bass_kernel_spmd

    nc = _get_nc()
    in_maps = _make_in_maps(np.asarray(x, np.float32), np.asarray(Wq, np.float32),
                            np.asarray(bq, np.float32), np.asarray(Wk, np.float32),
                            np.asarray(bk, np.float32), np.asarray(Wv, np.float32),
                            np.asarray(bv, np.float32), np.asarray(Wo, np.float32))
    res = run_bass_kernel_spmd(nc, in_maps, core_ids=list(range(N_CORES)))
    full = np.empty((B, S, D), np.float32)
    for b in range(B):
        acc = res.results[4 * b]["out"].astype(np.float32).copy()
        for g in range(1, GROUPS):
            acc += res.results[4 * b + g]["out"]
        full[b] = acc + np.asarray(bo, np.float32)[None, :]
    if _return_results:
        return full, res
    return full


# revision 13
# speedup vs baseline: 1.1637x; 1.1637x over previous
"""Causal multi-head attention (B=2, S=2048, D=1024, 16 heads of 64) on 8 TRN2
NeuronCores.

Sharding: core c -> batch b = c//4, head-group g = c%4 (4 heads = 256 model
dims per core).  Wq/Wk/Wv column-parallel, Wo row-parallel; the 4 partial
outputs per batch are summed on the host (no collectives).

Per-core data flow (matmul compute in bf16, fp32 PSUM accumulation):
  QT = (Wq_g/8) @ x^T      [256, 2048]   (1/sqrt(hd) folded into Wq,bq)
  KT = Wk_g @ x^T          [256, 2048]
  V  = x @ Wv_g^T + bv     [2048, 256]   natural layout, ones-augmented
  attention per head pair:
    ST[sk,sq] = K_h @ Q_h^T          two heads at row groups 0/64
    causal mask applied IN PSUM via a second accumulating matmul
        (identity stationary x (-3e4) upper-triangle moving tile)
    P = exp(ST)                      ACT exp, [128,1024] two heads wide
    AV: single K=128 matmul per (head, ki) accumulating a merged
        [65,1024] pos tile (hh halves side by side; ones-row of V gives l)
    evac: preoutT = pos[0:64] * partition_broadcast(1/l)  (l-row copied to
        a base-partition-0 tile first; custom DVE/gpsimd ops misread
        partition-offset APs on HW)
  out_partial = preoutT.T @ Wo_g^T   [2048, 1024] fp32
Host: out[b] = sum of the 4 head-group partials + bo.

Scheduling notes (from trace analysis of the previous version):
  - each dma_start costs ~600-900ns of sequencer time to configure the DGE;
    inputs are split across the TWO hwdge queues (sync: xT chunks,
    scalar: weights) so configs run in parallel and the first projection
    matmuls chase the per-chunk DMAs instead of waiting ~13us.
  - Q-t0/K-t0 projections are chunk-chased during the input load (Q in
    mm_ps, K in po_ps, 8 PSUM banks total); evacs split DVE/ACT.
  - PSUM pools: 2+2 bufs of [128,1024] fp32 (4KB) each.
"""

import numpy as np
import ml_dtypes

B, S, D = 2, 2048, 1024
HD = 64
NH = D // HD
N_CORES = 8
GROUPS = 4          # head-groups (tensor-parallel)
JG = D // GROUPS    # local dims per core = 256
NHL = JG // HD      # local heads = 4
KCH = D // 128      # contraction chunks for projections = 8
NKT = S // 128      # sk tiles = 16
NJB = S // 512      # query blocks of 512 = 4

BF16 = ml_dtypes.bfloat16

_cached = {}


def _build():
    import concourse.bacc as bacc
    import concourse.tile as tile
    import concourse.mybir as mybir

    f32 = mybir.dt.float32
    bf16 = mybir.dt.bfloat16
    Exp = mybir.ActivationFunctionType.Exp
    Identity = mybir.ActivationFunctionType.Identity

    nc = bacc.Bacc("TRN2", target_bir_lowering=False, debug=False,
                   num_devices=N_CORES)

    xT = nc.dram_tensor("xT", [128, KCH, S], bf16, kind="ExternalInput").ap()
    wqT = nc.dram_tensor("wqT", [128, KCH, JG], bf16, kind="ExternalInput").ap()
    wkT = nc.dram_tensor("wkT", [128, KCH, JG], bf16, kind="ExternalInput").ap()
    wvT = nc.dram_tensor("wvT", [128, KCH, JG], bf16, kind="ExternalInput").ap()
    woT = nc.dram_tensor("woT", [128, 2, D], bf16, kind="ExternalInput").ap()
    bqk = nc.dram_tensor("bqk", [128, 4], f32, kind="ExternalInput").ap()
    bvb = nc.dram_tensor("bvb", [128, JG], f32, kind="ExternalInput").ap()
    cmask = nc.dram_tensor("cmask", [128, 256], bf16, kind="ExternalInput").ap()
    out = nc.dram_tensor("out", [S, D], bf16, kind="ExternalOutput").ap()

    with tile.TileContext(nc) as tc:
        with (
            tc.tile_pool(name="const", bufs=1) as cpool,
            tc.tile_pool(name="pbig", bufs=2) as p_pool,
            tc.tile_pool(name="small", bufs=4) as small_pool,
            tc.tile_pool(name="outp", bufs=3) as out_pool,
            tc.tile_pool(name="mm_ps", bufs=2, space="PSUM") as mm_ps,
            tc.tile_pool(name="po_ps", bufs=2, space="PSUM") as po_ps,
        ):
            # ---- input DMAs, two hwdge queues in consumption order ----
            # scalar queue: weights.  wq/wk split per k-chunk so the
            # chunk-chasing projections only wait on their own chunk.
            wq_sb = cpool.tile([128, KCH, JG], bf16)
            wk_sb = cpool.tile([128, KCH, JG], bf16)
            for k in range(4):
                nc.scalar.dma_start(wq_sb[:, k, :], wqT[:, k, :])
            bqk_sb = cpool.tile([128, 4], f32)
            nc.scalar.dma_start(bqk_sb[:], bqk[:])
            cm_sb = cpool.tile([128, 256], bf16)
            nc.scalar.dma_start(cm_sb[:], cmask[:])
            for k in range(4, KCH):
                nc.scalar.dma_start(wq_sb[:, k, :], wqT[:, k, :])
            for k in range(KCH):
                nc.scalar.dma_start(wk_sb[:, k, :], wkT[:, k, :])
            # sync queue: the big xT chunks, then the late-needed weights
            xt_all = cpool.tile([128, KCH, S], bf16)
            for k in range(KCH):
                nc.sync.dma_start(xt_all[:, k, :], xT[:, k, :])
            wv_sb = cpool.tile([128, KCH, JG], bf16)
            nc.sync.dma_start(wv_sb[:], wvT[:])
            bvb_sb = cpool.tile([128, JG], f32)
            nc.sync.dma_start(bvb_sb[:], bvb[:])
            wo_sb = cpool.tile([128, 2, D], bf16)
            nc.sync.dma_start(wo_sb[:], woT[:])

            qt = [cpool.tile([128, S], bf16, name=f"qt{t}") for t in range(2)]
            kt = [cpool.tile([128, S], bf16, name=f"kt{t}") for t in range(2)]
            v_all = cpool.tile([128, NKT, NHL * 65], bf16)
            nc.vector.memset(
                v_all.rearrange("p k (h c) -> p k h c", c=65)[:, :, :, 64:65], 1.0)
            po = [cpool.tile([128, S], bf16, name=f"po{t}") for t in range(2)]
            ones64 = cpool.tile([1, 64], f32)
            nc.vector.memset(ones64[:], 1.0)
            warm = small_pool.tile([1, 4], f32, tag="lrow")
            nc.vector.memset(warm[:], 0.0)
            nc.scalar.activation(warm[:], warm[:], Exp)

            # ---- load-phase: Q-t0 / K-t0 chunk-chasing ----
            # 4 live [128,1024] PSUM tiles (all 8 banks); emission order is
            # arrival order: Q k=0..3 first (wk configs trail wq's on the
            # scalar queue), then interleaved K/Q.
            q_ps = [mm_ps.tile([128, 1024], f32, tag="mm", name=f"q0ps{n}")
                    for n in range(2)]
            k_ps = [po_ps.tile([128, 1024], f32, tag="po", name=f"k0ps{n}")
                    for n in range(2)]

            def chase(ps, w_sb_, k):
                for np_ in range(2):
                    for half in range(2):
                        n = 2 * np_ + half
                        nc.tensor.matmul(
                            ps[np_][:, 512 * half:512 * half + 512],
                            lhsT=w_sb_[:, k, 0:128],
                            rhs=xt_all[:, k, 512 * n:512 * n + 512],
                            start=(k == 0), stop=(k == KCH - 1))

            for k in range(4):
                chase(q_ps, wq_sb, k)
            for k in range(4):
                chase(k_ps, wk_sb, k)
                chase(q_ps, wq_sb, k + 4)
            for k in range(4, KCH):
                chase(k_ps, wk_sb, k)
            # evacs: Q on DVE, K on ACT (parallel); np0 first so attention
            # (which needs cols 0:512 of qt/kt) unblocks earliest.
            nc.vector.tensor_scalar_add(qt[0][:, 0:1024], q_ps[0][:],
                                        bqk_sb[:, 0:1])
            nc.scalar.activation(kt[0][:, 0:1024], k_ps[0][:], Identity,
                                 bias=bqk_sb[:, 2:3])
            nc.vector.tensor_scalar_add(qt[0][:, 1024:2048], q_ps[1][:],
                                        bqk_sb[:, 0:1])
            nc.scalar.activation(kt[0][:, 1024:2048], k_ps[1][:], Identity,
                                 bias=bqk_sb[:, 2:3])

            # ---- phase helpers ----

            def proj_qkt(w_sb_, bcol, dst, t):
                for np_ in range(2):
                    ps = mm_ps.tile([128, 1024], f32, tag="mm",
                                    name=f"psproj{t}_{np_}")
                    for k in range(KCH):
                        for half in range(2):
                            n = 2 * np_ + half
                            nc.tensor.matmul(
                                ps[:, 512 * half:512 * half + 512],
                                lhsT=w_sb_[:, k, 128 * t:128 * t + 128],
                                rhs=xt_all[:, k, 512 * n:512 * n + 512],
                                start=(k == 0), stop=(k == KCH - 1))
                    nc.vector.tensor_scalar_add(
                        dst[t][:, 1024 * np_:1024 * np_ + 1024], ps[:],
                        bqk_sb[:, bcol + t:bcol + t + 1])

            def proj_v(sg, pool=None):
                # two s-tiles per [128,1024] tile, one per bank
                pool = pool or mm_ps
                ps = pool.tile([128, 1024], f32,
                               tag="po" if pool is po_ps else "mm",
                               name=f"psv{sg}")
                for k in range(KCH):
                    for q in range(2):
                        si = 2 * sg + q
                        nc.tensor.matmul(
                            ps[:, 512 * q:512 * q + 256],
                            lhsT=xt_all[:, k, 128 * si:128 * si + 128],
                            rhs=wv_sb[:, k, :],
                            start=(k == 0), stop=(k == KCH - 1))
                for q in range(2):
                    si = 2 * sg + q
                    nc.vector.tensor_add(
                        v_all[:, si, :].rearrange(
                            "p (h c) -> p h c", c=65)[:, :, 0:64],
                        ps[:, 512 * q:512 * q + 256].rearrange(
                            "p (h c) -> p h c", c=64),
                        bvb_sb.rearrange("p (h c) -> p h c", c=64))

            def attn_scores(pair, j, interleave=None):
                nk = 4 * (j + 1)
                qt_t, kt_t = qt[pair], kt[pair]
                pt_all = p_pool.tile([128, NKT, 1024], bf16, tag="p",
                                     name=f"pt{pair}_{j}")
                for ki in range(nk):
                    d = max(0, 128 * ki - 512 * j)
                    st = mm_ps.tile([128, 1024], f32, tag="mm",
                                    name=f"st{pair}_{j}_{ki}")
                    diag = ki >= 4 * j
                    for hh in range(2):
                        base = 64 * hh
                        nc.tensor.matmul(
                            st[:, 512 * hh + d:512 * hh + 512],
                            lhsT=kt_t[base:base + 64,
                                      128 * ki:128 * ki + 128],
                            rhs=qt_t[base:base + 64,
                                     512 * j + d:512 * j + 512],
                            start=True, stop=not diag,
                            skip_group_check=diag)
                    if diag:
                        # accumulate -3e4 into the strictly-lower-triangle
                        # (sk>sq) of the 128-wide diagonal sub-block, so exp
                        # gives exactly 0 there: identity (stationary) x
                        # maskneg (moving).
                        for hh in range(2):
                            nc.tensor.matmul(
                                st[:, 512 * hh + d:512 * hh + d + 128],
                                lhsT=cm_sb[:, 0:128],
                                rhs=cm_sb[:, 128:256],
                                start=False, stop=True,
                                skip_group_check=True)
                    if d == 0:
                        nc.scalar.activation(pt_all[:, ki, :], st[:], Exp)
                    else:
                        for hh in range(2):
                            nc.scalar.activation(
                                pt_all[:, ki, 512 * hh + d:512 * hh + 512],
                                st[:, 512 * hh + d:512 * hh + 512], Exp)
                    if interleave is not None:
                        interleave(ki)
                return pt_all

            def attn_av(pair, j, pt_all, final=False):
                nk = 4 * (j + 1)
                # AV: single K=128 matmul per (head, ki) — matmul wall time
                # depends only on the moving size N.  Both heads accumulate
                # into one [65,1024] tile (hh halves side by side).
                pos = po_ps.tile([65, 1024], f32, tag="po",
                                 name=f"pos{pair}_{j}")
                for ki in range(nk):
                    d = max(0, 128 * ki - 512 * j)
                    for hh in range(2):
                        h = 2 * pair + hh
                        nc.tensor.matmul(
                            pos[0:65, 512 * hh + d:512 * hh + 512],
                            lhsT=v_all[:, ki, 65 * h:65 * h + 65],
                            rhs=pt_all[:, ki, 512 * hh + d:512 * hh + 512],
                            start=(ki == 0), stop=(ki == nk - 1),
                            skip_group_check=True)
                # evac: normalize by 1/l. The l-row must be copied to a
                # base-partition-0 tile first — the custom DVE/gpsimd ops
                # (recip, partition_broadcast) misread partition-offset APs
                # on HW.  Ops stay 512-wide per head so the chain pipelines.
                lrow = [small_pool.tile([1, 512], f32, tag="lrow",
                                        name=f"lrow{hh}") for hh in range(2)]
                lrec = [small_pool.tile([1, 512], f32, tag="lrec",
                                        name=f"lrec{hh}") for hh in range(2)]
                for hh in range(2):
                    nc.vector.tensor_copy(lrow[hh][:],
                                          pos[64:65, 512 * hh:512 * hh + 512])
                    nc.vector.reciprocal_approx_fast(lrec[hh][:], lrow[hh][:])
                rb = [small_pool.tile([64, 512], f32, tag="rb",
                                      name=f"rb{hh}") for hh in range(2)]
                for hh in range(2):
                    nc.gpsimd.partition_broadcast(rb[hh][:], lrec[hh][:])
                for hh in range(2):
                    nc.vector.tensor_mul(
                        po[pair][64 * hh:64 * hh + 64,
                                 512 * j:512 * j + 512],
                        pos[0:64, 512 * hh:512 * hh + 512],
                        rb[hh][:])

            def wo_tile(m, evac_engine=None, pool=None):
                pool = pool or mm_ps
                ps = pool.tile([128, 1024], f32,
                               tag="po" if pool is po_ps else "mm",
                               name=f"pswo{m}")
                for t in range(2):
                    for n in range(2):
                        nc.tensor.matmul(
                            ps[:, 512 * n:512 * n + 512],
                            lhsT=po[t][:, 128 * m:128 * m + 128],
                            rhs=wo_sb[:, t, 512 * n:512 * n + 512],
                            start=(t == 0), stop=(t == 1))
                ob = out_pool.tile([128, 1024], bf16, tag="ob")
                if evac_engine == "scalar":
                    nc.scalar.copy(ob[:], ps[:])
                else:
                    nc.vector.tensor_copy(ob[:], ps[:])
                nc.sync.dma_start(out[128 * m:128 * m + 128, :], ob[:])

            def wo_block(wj, alternate=False):
                # scalar-engine evacs only where ACT has slack (the final
                # block); elsewhere they would stall the exp stream.
                for m in range(4 * wj, 4 * wj + 4):
                    wo_tile(m, evac_engine="scalar" if alternate and m % 2
                            else None)

            # ---- fused software pipeline ----
            # Per ki the PE emits the NEXT block's scores matmuls (+causal
            # mask) and the CURRENT block's AV matmuls (~0.97us) while ACT
            # runs one exp (~0.95us): the two engines stay locally balanced
            # instead of ping-ponging between scores-phases (ACT-bound) and
            # AV-phases (PE-bound).  V projections and wo tiles fill the ki
            # slots where the AV stream is shorter, using the po_ps buffer
            # freed by the previous block's evac.

            def emit_scores_ki(pair, j, ki, pt_all):
                d = max(0, 128 * ki - 512 * j)
                st = mm_ps.tile([128, 1024], f32, tag="mm",
                                name=f"st{pair}_{j}_{ki}")
                diag = ki >= 4 * j
                for hh in range(2):
                    base = 64 * hh
                    nc.tensor.matmul(
                        st[:, 512 * hh + d:512 * hh + 512],
                        lhsT=kt[pair][base:base + 64,
                                      128 * ki:128 * ki + 128],
                        rhs=qt[pair][base:base + 64,
                                     512 * j + d:512 * j + 512],
                        start=True, stop=not diag,
                        skip_group_check=diag)
                if diag:
                    for hh in range(2):
                        nc.tensor.matmul(
                            st[:, 512 * hh + d:512 * hh + d + 128],
                            lhsT=cm_sb[:, 0:128],
                            rhs=cm_sb[:, 128:256],
                            start=False, stop=True,
                            skip_group_check=True)
                if d == 0:
                    nc.scalar.activation(pt_all[:, ki, :], st[:], Exp)
                else:
                    for hh in range(2):
                        nc.scalar.activation(
                            pt_all[:, ki, 512 * hh + d:512 * hh + 512],
                            st[:, 512 * hh + d:512 * hh + 512], Exp)

            def emit_av_ki(pair, j, ki, pt_all, pos, nk):
                d = max(0, 128 * ki - 512 * j)
                for hh in range(2):
                    h = 2 * pair + hh
                    nc.tensor.matmul(
                        pos[0:65, 512 * hh + d:512 * hh + 512],
                        lhsT=v_all[:, ki, 65 * h:65 * h + 65],
                        rhs=pt_all[:, ki, 512 * hh + d:512 * hh + 512],
                        start=(ki == 0), stop=(ki == nk - 1),
                        skip_group_check=True)

            def emit_av_evac(pair, j, pos):
                lrow = [small_pool.tile([1, 512], f32, tag="lrow",
                                        name=f"lrow{hh}") for hh in range(2)]
                lrec = [small_pool.tile([1, 512], f32, tag="lrec",
                                        name=f"lrec{hh}") for hh in range(2)]
                for hh in range(2):
                    nc.vector.tensor_copy(lrow[hh][:],
                                          pos[64:65, 512 * hh:512 * hh + 512])
                    nc.vector.reciprocal_approx_fast(lrec[hh][:], lrow[hh][:])
                rb = [small_pool.tile([64, 512], f32, tag="rb",
                                      name=f"rb{hh}") for hh in range(2)]
                for hh in range(2):
                    nc.gpsimd.partition_broadcast(rb[hh][:], lrec[hh][:])
                for hh in range(2):
                    nc.vector.tensor_mul(
                        po[pair][64 * hh:64 * hh + 64,
                                 512 * j:512 * j + 512],
                        pos[0:64, 512 * hh:512 * hh + 512],
                        rb[hh][:])

            # early segment: first block + t=1 projections
            proj_v(0)
            proj_v(1)
            pt00 = p_pool.tile([128, NKT, 1024], bf16, tag="p", name="pt0_0")
            for ki in range(4):
                emit_scores_ki(0, 0, ki, pt00)
            pos00 = po_ps.tile([65, 1024], f32, tag="po", name="pos0_0")
            proj_qkt(wq_sb, 0, qt, 1)
            for ki in range(4):
                emit_av_ki(0, 0, ki, pt00, pos00, 4)
            emit_av_evac(0, 0, pos00)
            proj_qkt(wk_sb, 2, kt, 1)

            pts = {(0, 0): pt00}
            windows = [
                dict(s=(1, 0), fills=[lambda: proj_v(2, po_ps),
                                      lambda: proj_v(3, po_ps)]),
                dict(s=(0, 1), a=(1, 0), fills=[lambda: proj_v(4, po_ps),
                                                lambda: proj_v(5, po_ps)]),
                dict(s=(1, 1), a=(0, 1)),
                dict(s=(0, 2), a=(1, 1), fills=[lambda: proj_v(6, po_ps),
                                                lambda: proj_v(7, po_ps)]),
                dict(s=(1, 2), a=(0, 2),
                     fills=[lambda: wo_tile(0, pool=po_ps),
                            lambda: wo_tile(1, pool=po_ps)]),
                dict(s=(0, 3), a=(1, 2),
                     fills=[lambda: wo_tile(2, pool=po_ps),
                            lambda: wo_tile(3, pool=po_ps)]),
                dict(s=(1, 3), a=(0, 3),
                     fills=[lambda: wo_tile(4, pool=po_ps)]),
            ]
            # run the fused windows, threading pt tiles through
            for w in windows:
                sp = w.get('s')
                ap = w.get('a')
                fills = list(w.get('fills', ()))
                nkS = 4 * (sp[1] + 1) if sp else 0
                nkA = 4 * (ap[1] + 1) if ap else 0
                if sp:
                    pts[sp] = p_pool.tile([128, NKT, 1024], bf16, tag="p",
                                          name=f"pt{sp[0]}_{sp[1]}")
                pos = None
                if ap:
                    pos = po_ps.tile([65, 1024], f32, tag="po",
                                     name=f"pos{ap[0]}_{ap[1]}")
                for i in range(max(nkS, nkA)):
                    if i < nkS:
                        emit_scores_ki(sp[0], sp[1], i, pts[sp])
                    if i < nkA:
                        emit_av_ki(ap[0], ap[1], i, pts[ap], pos, nkA)
                        if i == nkA - 1:
                            emit_av_evac(ap[0], ap[1], pos)
                    elif fills:
                        fills.pop(0)()
                for f in fills:
                    f()

            # tail: A13 interleaved with the remaining wo tiles
            pos13 = po_ps.tile([65, 1024], f32, tag="po", name="pos1_3")
            wo_rest = iter(range(5, 12))
            for ki in range(16):
                emit_av_ki(1, 3, ki, pts[(1, 3)], pos13, 16)
                if ki % 2 == 1:
                    m = next(wo_rest, None)
                    if m is not None:
                        wo_tile(m)
            emit_av_evac(1, 3, pos13)
            for m in list(wo_rest):
                wo_tile(m)
            wo_block(3, alternate=True)

    nc.compile()
    return nc


def _get_nc():
    if "nc" not in _cached:
        _cached["nc"] = _build()
    return _cached["nc"]


def _make_in_maps(x, Wq, bq, Wk, bk, Wv, bv, Wo):
    sc = 1.0 / np.sqrt(HD)
    tri = np.arange(128)
    ident = np.eye(128, dtype=np.float32)
    maskneg = np.where(tri[:, None] > tri[None, :], -30000.0, 0.0)
    cm = np.concatenate([ident, maskneg], axis=1).astype(BF16)
    in_maps = []
    for c in range(N_CORES):
        b, g = divmod(c, GROUPS)
        sl = slice(JG * g, JG * (g + 1))
        def tile_k(a):  # [D, M] -> [128, D//128, M] contiguous
            return np.ascontiguousarray(
                a.reshape(a.shape[0] // 128, 128, a.shape[1]).transpose(1, 0, 2))

        bqs = (bq[sl] * sc).astype(np.float32)
        bks = bk[sl].astype(np.float32)
        in_maps.append({
            "xT": tile_k(x[b].T.astype(BF16)),
            "wqT": tile_k((Wq[sl] * sc).T.astype(BF16)),
            "wkT": tile_k(Wk[sl].T.astype(BF16)),
            "wvT": tile_k(Wv[sl].T.astype(BF16)),
            "woT": tile_k(Wo[:, sl].T.astype(BF16)),
            "bqk": np.stack([bqs[0:128], bqs[128:256],
                             bks[0:128], bks[128:256]], axis=1).copy(),
            "bvb": np.broadcast_to(bv[sl].astype(np.float32), (128, JG)).copy(),
            "cmask": cm,
        })
    return in_maps


def kernel(x, Wq, bq, Wk, bk, Wv, bv, Wo, bo, _return_results=False):
    from concourse.bass_utils import run_bass_kernel_spmd

    nc = _get_nc()
    in_maps = _make_in_maps(np.asarray(x, np.float32), np.asarray(Wq, np.float32),
                            np.asarray(bq, np.float32), np.asarray(Wk, np.float32),
                            np.asarray(bk, np.float32), np.asarray(Wv, np.float32),
                            np.asarray(bv, np.float32), np.asarray(Wo, np.float32))
    res = run_bass_kernel_spmd(nc, in_maps, core_ids=list(range(N_CORES)))
    full = np.empty((B, S, D), np.float32)
    for b in range(B):
        acc = res.results[4 * b]["out"].astype(np.float32).copy()
        for g in range(1, GROUPS):
            acc += res.results[4 * b + g]["out"]
        full[b] = acc + np.asarray(bo, np.float32)[None, :]
    if _return_results:
        return full, res
    return full
